# revision 2
# baseline (speedup 1.0000x reference)
"""TRN2 Bass kernel v4: out = inp @ weights + bias.

Full shapes: inp [16384, 4096] f32, weights [4096, 8192] f32,
bias [8192] f32 -> out [16384, 8192] f32.

v4 = v2 topology (x-stationary, 8x1 data-parallel grid, mixed
bf16 + fp8e4-DoubleRow split-K) + DMA layout fixes driven by
microbenchmarks that showed per-queue DMA bandwidth is segment-size
bound (~42 GB/s @1KB, ~92 GB/s @4KB segments, rising with size):

  - W and x are PRE-TILED on the host into the exact SBUF tile layout
    ([group, partition, ksub, col]), so each W-group DMA reads 24KB
    contiguous per partition (~140 GB/s expected) instead of 1KB
    segments. 16 W-group DMAs instead of strided gathers.
  - Output written as [g, mq, p, 4, 512] tiles (4KB per-partition
    lines); host un-scrambles. 64 output DMAs of 512KB.
  - Queues: W (scalar), x + out (sync); W prefetched one group ahead.
  - Per-core GEMM: M=2048, K=4096 (24 bf16 + 8 fp8 K-subtiles),
    N=8192 in 16 groups of 512. PSUM [128,512] per (g, ms), 8-bank
    ring. Evacuation alternates DVE/ACT with 2^-16 scale (W is
    host-scaled by 2^16 to clear fp8e4's subnormal floor).
"""
import sys

sys.path.insert(0, "/opt/trn_rl_repo")

import numpy as np

B, F, C = 16384, 4096, 8192
P = 128
NCORES = 8
M_CORE = B // NCORES            # 2048
KS_ALL = F // P                 # 32
NG = 512                        # N-group width
N_GRPS = C // NG                # 16
XC = 512                        # x chunk (cols)
N_XC = M_CORE // XC             # 4
M_SUBS = M_CORE // P            # 16


def build(fp8_ks=8, w_bufs=2, out_bufs=4, scale_bits=16, compile_hw=True,
          body_reps=1, evac_split=True):
    import concourse.mybir as mybir
    import concourse.tile as tile
    from concourse import bacc
    from concourse.bass_interp import get_hw_module

    k1s = KS_ALL - fp8_ks
    k2s = fp8_ks
    assert k2s % 2 == 0

    nc = bacc.Bacc("TRN2", target_bir_lowering=False, debug=False,
                   num_devices=NCORES)

    bf16 = mybir.dt.bfloat16
    fp8 = mybir.dt.float8e4
    x16_dram = nc.dram_tensor("x16t", [N_XC, P, k1s, XC], bf16,
                              kind="ExternalInput")
    w16_dram = nc.dram_tensor("w16t", [N_GRPS, P, k1s, NG], bf16,
                              kind="ExternalInput")
    if k2s:
        x8_dram = nc.dram_tensor("x8t", [N_XC, P, k2s, XC], fp8,
                                 kind="ExternalInput")
        w8_dram = nc.dram_tensor("w8t", [N_GRPS, P, k2s, NG], fp8,
                                 kind="ExternalInput")
    out_dram = nc.dram_tensor("outt", [N_GRPS, M_SUBS // 4, P, 4, NG], bf16,
                              kind="ExternalOutput")

    scale = 2.0 ** -scale_bits if k2s else None

    with tile.TileContext(nc) as tc:
        with tc.tile_pool(name="xp", bufs=1) as x_pool, \
             tc.tile_pool(name="wp", bufs=w_bufs) as w_pool, \
             tc.tile_pool(name="outp", bufs=out_bufs) as out_pool, \
             tc.tile_pool(name="ps", bufs=8, space="PSUM") as psum_pool:
            for _rep in range(body_reps):
                x16_t = x_pool.tile([P, k1s, M_CORE], bf16, tag="x16",
                                    name="x16")
                x8_t = None
                if k2s:
                    x8_t = x_pool.tile([P, k2s, M_CORE], fp8, tag="x8",
                                       name="x8")
                for c in range(N_XC):
                    sl = slice(c * XC, (c + 1) * XC)
                    nc.sync.dma_start(x16_t[:, :, sl], x16_dram.ap()[c])
                    if k2s:
                        nc.sync.dma_start(x8_t[:, :, sl], x8_dram.ap()[c])

                def load_w(g):
                    wt16 = w_pool.tile([P, k1s, NG], bf16, tag="w16",
                                       name="w16")
                    nc.scalar.dma_start(wt16[:], w16_dram.ap()[g])
                    wt8 = None
                    if k2s:
                        wt8 = w_pool.tile([P, k2s, NG], fp8, tag="w8",
                                          name="w8")
                        nc.scalar.dma_start(wt8[:], w8_dram.ap()[g])
                    return wt16, wt8

                w_cur = load_w(0)
                for g in range(N_GRPS):
                    w_nxt = load_w(g + 1) if g + 1 < N_GRPS else None
                    wt16, wt8 = w_cur
                    ot = None
                    for ms in range(M_SUBS):
                        mo = ms * P
                        if ms % 4 == 0:
                            ot = out_pool.tile([P, 4, NG], bf16, tag="ot",
                                               name="ot")
                        ps = psum_pool.tile([P, NG], mybir.dt.float32,
                                            tag="ps", name="ps")
                        for ks in range(k1s):
                            nc.tensor.matmul(
                                ps, x16_t[:, ks, mo:mo + P],
                                wt16[:, ks, :],
                                start=(ks == 0),
                                stop=(k2s == 0 and ks == k1s - 1),
                            )
                        for j in range(k2s // 2):
                            nc.tensor.matmul(
                                ps, x8_t[:, 2 * j:2 * j + 2, mo:mo + P],
                                wt8[:, 2 * j:2 * j + 2, :],
                                start=(k1s == 0 and j == 0),
                                stop=(j == k2s // 2 - 1),
                                perf_mode=mybir.MatmulPerfMode.DoubleRow,
                            )
                        osl = ot[:, ms % 4, :]
                        use_scalar = evac_split and (ms % 2 == 1)
                        if k2s:
                            if use_scalar:
                                nc.scalar.mul(osl, ps[:], scale)
                            else:
                                nc.vector.tensor_scalar_mul(osl, ps[:],
                                                            scale)
                        else:
                            if use_scalar:
                                nc.scalar.copy(osl, ps[:])
                            else:
                                nc.vector.tensor_copy(osl, ps[:])
                        if ms % 4 == 3:
                            nc.sync.dma_start(out_dram.ap()[g, ms // 4],
                                              ot[:])
                    w_cur = w_nxt

    nc.compile()
    if compile_hw:
        nc.m = get_hw_module(nc.m)
    return nc


_compiled = None
_last_in_maps = None
CFG = {}


def _transpose(a: np.ndarray) -> np.ndarray:
    try:
        import torch

        return torch.from_numpy(np.ascontiguousarray(a)).t().contiguous().numpy()
    except ImportError:
        r, c = a.shape
        bs = 128
        out = np.empty((c, r), a.dtype)
        v = a.reshape(r // bs, bs, c // bs, bs)
        o = out.reshape(c // bs, bs, r // bs, bs)
        np.copyto(o, v.transpose(2, 3, 0, 1))
        return out


def _tile4(a: np.ndarray, width: int) -> np.ndarray:
    """[ks*128, cols] -> [cols//width, 128, ks, width] contiguous."""
    ks = a.shape[0] // P
    nt = a.shape[1] // width
    return np.ascontiguousarray(
        a.reshape(ks, P, nt, width).transpose(2, 1, 0, 3))


def kernel(inp: np.ndarray, weights: np.ndarray, bias: np.ndarray) -> np.ndarray:
    global _compiled, _last_in_maps
    import ml_dtypes
    from concourse import bass_utils

    cfg = dict(fp8_ks=8, scale_bits=16)
    cfg.update(CFG)
    if _compiled is None:
        _compiled = build(**cfg)
    nc = _compiled

    fp8_ks = cfg["fp8_ks"]
    sb = cfg["scale_bits"]
    K1 = F - fp8_ks * P
    s = np.float32(2.0 ** sb) if fp8_ks else np.float32(1.0)

    inp = np.ascontiguousarray(inp, dtype=np.float32)
    weights = np.ascontiguousarray(weights, dtype=np.float32)
    inpT = _transpose(inp)                               # [F, B] f32
    x16_all = inpT[:K1].astype(ml_dtypes.bfloat16)
    w16t = _tile4((weights[:K1] * s).astype(ml_dtypes.bfloat16), NG)
    if fp8_ks:
        x8_all = inpT[K1:].astype(ml_dtypes.float8_e4m3fn)
        w8t = _tile4((weights[K1:] * s).astype(ml_dtypes.float8_e4m3fn), NG)

    in_maps = []
    for c in range(NCORES):
        sl = slice(c * M_CORE, (c + 1) * M_CORE)
        m = {
            "x16t": _tile4(x16_all[:, sl], XC),
            "w16t": w16t,
        }
        if fp8_ks:
            m["x8t"] = _tile4(x8_all[:, sl], XC)
            m["w8t"] = w8t
        in_maps.append(m)

    _last_in_maps = in_maps
    res = bass_utils.run_bass_kernel_spmd(nc, in_maps, list(range(NCORES)))

    out = np.empty((B, C), np.float32)
    bias32 = bias.astype(np.float32, copy=False)
    for c in range(NCORES):
        blk = out[c * M_CORE:(c + 1) * M_CORE]
        r = res.results[c]["outt"]          # [16, 4, 128, 4, 512] bf16
        r = r.transpose(1, 3, 2, 0, 4).reshape(M_CORE, C)
        np.copyto(blk, r.astype(np.float32))
        blk += bias32[None, :]
    return out
